# revision 25
# baseline (speedup 1.0000x reference)
"""CenterReppHead on 8 Trainium2 NeuronCores (Bass/Tile).

Sharding: data-parallel over (batch=2) x (4 H-chunks of 32 rows) = 8 cores.
Each core computes the full network on a haloed row-slab:
  x slab      44 rows  [h0-6, h1+6)
  conv1 out   42 rows
  conv2 out   40 rows
  tower feat  38 rows  [h0-3, h1+3)   (covers DCN bilinear sampling, |offset|<1)
Slabs use 132-wide rows (2 zero pad cols each side). Tower features are
spilled to DRAM and reloaded (SBUF phase budget).

DCN: out_T[pix, oc] = sum_k sum_{4 corners} w_corner(pix) * (gathered_corner^T @ W_k)
  - per-pixel bilinear corner weights commute past the channel contraction
  - gathers on GPSIMD ap_gather (indices shared across channel partitions)
  - weights applied post-matmul via scalar_tensor_tensor (per-partition MAC)
"""

import sys

if "/opt/trn_rl_repo" not in sys.path:
    sys.path.insert(0, "/opt/trn_rl_repo")

from contextlib import ExitStack

import numpy as np

import concourse.bass as bass
import concourse.bacc as bacc
import concourse.tile as tile
import concourse.mybir as mybir
from concourse.bass_utils import run_bass_kernel_spmd

F32 = mybir.dt.float32
I16 = mybir.dt.int16
I32 = mybir.dt.int32
AF = mybir.ActivationFunctionType
OP = mybir.AluOpType

B, C, H, W = 2, 256, 128, 128
NCLS, NP = 80, 9
ROWS = 32
N_CORES = 8
WP = 132
RX, R1, R2, RF = 44, 42, 40, 38
NE = RF * WP
CHUNK = 4
FC = CHUNK * 128          # 512
NCH = ROWS // CHUNK       # 8

PERM27 = [2 * k for k in range(9)] + [2 * k + 1 for k in range(9)] + list(range(18, 27))
PERM18 = [2 * k for k in range(9)] + [2 * k + 1 for k in range(9)]

TOWER_W = ["w0c", "w1c", "w2c", "w0r", "w1r", "w2r", "wic"]


def build_program_flat():
    nc = bacc.Bacc("TRN2", target_bir_lowering=False, debug=False,
                   num_devices=N_CORES)

    def din(name, shape, dt=F32):
        return nc.dram_tensor(name, shape, dt, kind="ExternalInput").ap()

    xs = din("xs", [2, 128, RX, WP])
    wts = {n: din(n, [128, 2, 2, 9, 128]) for n in TOWER_W}
    wdc = din("wdc", [128, 2, 9, 256])
    wdr = din("wdr", [128, 2, 9, 256])
    wio = din("wio", [128, 2, 27])
    wco = din("wco", [128, 2, 80])
    wro = din("wro", [128, 2, 18])
    bt = din("bt", [128, 14])
    bio = din("bio", [27, 1])
    bco = din("bco", [128, 80])
    brop = din("brop", [9, 2])
    kyc = din("kyc", [9, 1])
    kxc = din("kxc", [9, 1])
    cidx = din("cidx", [9, 1])
    rrow = din("rrow", [9, ROWS * 128])
    wcol = din("wcol", [9, FC])
    ident = din("ident", [128, 128])
    mrow = din("mrow", [128, 24])

    co = nc.dram_tensor("co", [ROWS * 128, 80], F32, kind="ExternalOutput").ap()
    wr = nc.dram_tensor("wr", [ROWS * 128, 4], F32, kind="ExternalOutput").ap()

    # DRAM spill buffers for the two tower features (no pad cols)
    cfD = nc.dram_tensor("cfD", [2, 128, RF, 128], F32).ap()
    pfD = nc.dram_tensor("pfD", [2, 128, RF, 128], F32).ap()
    offcD = nc.dram_tensor("offcD", [27, ROWS * 128], F32).ap()

    with tile.TileContext(nc) as tc, ExitStack() as ctx:
        sing = ctx.enter_context(tc.tile_pool(name="sing", bufs=1))
        wrot = ctx.enter_context(tc.tile_pool(name="wrot", bufs=2))
        dramp = ctx.enter_context(tc.tile_pool(name="dramp", bufs=2, space="DRAM"))

        ld = nc.sync.dma_start
        tsc = nc.vector.tensor_scalar
        ttn = nc.vector.tensor_tensor
        stt = nc.vector.scalar_tensor_tensor

        def sload(name, shape, ap_, dt=F32):
            t = sing.tile(shape, dt, tag=name, name=name)
            ld(out=t, in_=ap_)
            return t

        bt_sb = sload("btS", [128, 14], bt)
        bio_sb = sload("bioS", [27, 1], bio)
        bco_sb = sload("bcoS", [128, 80], bco)
        brop_sb = sload("bropS", [9, 2], brop)
        kyc_sb = sload("kycS", [9, 1], kyc)
        kxc_sb = sload("kxcS", [9, 1], kxc)
        cidx_sb = sload("cidxS", [9, 1], cidx)
        wcol_sb = sload("wcolS", [9, FC], wcol)
        ident_sb = sload("identS", [128, 128], ident)
        wio_sb = sload("wioS", [128, 2, 27], wio)
        wco_sb = sload("wcoS", [128, 2, 80], wco)
        wro_sb = sload("wroS", [128, 2, 18], wro)
        mrow_sb = sload("mrowS", [128, 24], mrow)

        wrb = sing.tile([128, ROWS, 4], F32, tag="wrb", name="wrb")

        def load_w(name, shape=(128, 2, 2, 9, 128), src=None):
            t = wrot.tile(list(shape), F32, tag="w", name=f"w_{name}")
            ld(out=t, in_=src if src is not None else wts[name])
            return t

        def conv_stage(pool_ps, src2, wt, bcol0, dst_write, rout, r_off):
            """dst_write(och, rg, nr, psum_view) consumes the relu+bias output."""
            n_rg = (rout + 3) // 4
            for och in range(2):
                for rg in range(n_rg):
                    r0 = rg * 4
                    nr = min(4, rout - r0)
                    ps = pool_ps.tile([128, 4, 128], F32, tag="pconv", name="pconv")
                    for ich in range(2):
                        for kpt in range(9):
                            ky, kx = kpt // 3, kpt % 3
                            rhs = src2[ich][:, r0 + r_off + ky: r0 + r_off + ky + nr,
                                            kx + 1: kx + 129]
                            nc.tensor.matmul(ps[:, :nr, :],
                                             lhsT=wt[:, ich, och, kpt, :], rhs=rhs,
                                             start=(ich == 0 and kpt == 0),
                                             stop=(ich == 1 and kpt == 8))
                    dst_write(och, r0, nr, ps[:, :nr, :],
                              bt_sb[:, bcol0 + och: bcol0 + och + 1])

        def to_slab(dst2):
            def w(och, r0, nr, psv, bias):
                nc.scalar.activation(out=dst2[och][:, r0:r0 + nr, 2:130], in_=psv,
                                     func=AF.Relu, bias=bias, scale=1.0)
            return w

        def mask_rows(slab2, rtot, m, col0):
            # zero rows that fall outside the image (per-core 0/1 in mrow)
            for hh in range(2):
                for j in range(m):
                    tsc(slab2[hh][:, j, :], slab2[hh][:, j, :],
                        mrow_sb[:, col0 + j: col0 + j + 1], None, OP.mult)
                for j in range(m):
                    tsc(slab2[hh][:, rtot - m + j, :], slab2[hh][:, rtot - m + j, :],
                        mrow_sb[:, col0 + m + j: col0 + m + j + 1], None, OP.mult)

        # ================= phase A: towers (features spilled to DRAM) ========
        with tc.tile_pool(name="towp", bufs=1) as towp, \
                tc.tile_pool(name="pA", bufs=2, space="PSUM") as pA, \
                tc.tile_pool(name="stgp", bufs=3) as stgp:

            xsb = []
            for hh in range(2):
                t = towp.tile([128, RX, WP], F32, tag=f"x{hh}", name=f"x{hh}")
                ld(out=t, in_=xs[hh])
                xsb.append(t)

            def spill(dstD):
                def w(och, r0, nr, psv, bias):
                    st = stgp.tile([128, 4, 128], F32, tag="stg", name="stg")
                    nc.scalar.activation(out=st[:, :nr, :], in_=psv, func=AF.Relu,
                                         bias=bias, scale=1.0)
                    ld(out=dstD[och][:, r0:r0 + nr, :], in_=st[:, :nr, :])
                return w

            for tw in range(2):  # 0 = cls, 1 = reg
                s1 = [towp.tile([128, R1, WP], F32, tag=f"s1{hh}", name=f"s1{hh}")
                      for hh in range(2)]
                for hh in range(2):
                    nc.vector.memset(s1[hh][:, :, 0:2], 0.0)
                    nc.vector.memset(s1[hh][:, :, 130:132], 0.0)
                conv_stage(pA, xsb, load_w(TOWER_W[3 * tw]), 6 * tw, to_slab(s1),
                           R1, 0)
                mask_rows(s1, R1, 5, 0)
                s2 = [towp.tile([128, R2, WP], F32, tag=f"s2{hh}", name=f"s2{hh}")
                      for hh in range(2)]
                for hh in range(2):
                    nc.vector.memset(s2[hh][:, :, 0:2], 0.0)
                    nc.vector.memset(s2[hh][:, :, 130:132], 0.0)
                conv_stage(pA, s1, load_w(TOWER_W[3 * tw + 1]), 6 * tw + 2,
                           to_slab(s2), R2, 0)
                mask_rows(s2, R2, 4, 10)
                conv_stage(pA, s2, load_w(TOWER_W[3 * tw + 2]), 6 * tw + 4,
                           spill(cfD if tw == 0 else pfD), RF, 0)

        # ================= phase A2: reload pf, init head ====================
        pfp = ctx.enter_context(tc.tile_pool(name="pfp", bufs=1))
        pf = []
        for hh in range(2):
            t = pfp.tile([128, RF, WP], F32, tag=f"pf{hh}", name=f"pf{hh}")
            nc.vector.memset(t[:, :, 0:2], 0.0)
            nc.vector.memset(t[:, :, 130:132], 0.0)
            ld(out=t[:, :, 2:130], in_=pfD[hh])
            pf.append(t)
        mask_rows(pf, RF, 3, 18)

        with tc.tile_pool(name="initp", bufs=1) as initp, \
                tc.tile_pool(name="pA2", bufs=2, space="PSUM") as pA2:
            ic = [initp.tile([128, ROWS, 128], F32, tag=f"ic{hh}", name=f"ic{hh}")
                  for hh in range(2)]

            def ic_write(och, r0, nr, psv, bias):
                nc.scalar.activation(out=ic[och][:, r0:r0 + nr, :], in_=psv,
                                     func=AF.Relu, bias=bias, scale=1.0)

            conv_stage(pA2, pf, load_w("wic"), 12, ic_write, ROWS, 2)

            for rg in range(8):
                ps27 = pA2.tile([27, 4, 128], F32, tag="pio", name="pio")
                for ich in range(2):
                    rnat = ic[ich][:, rg * 4:rg * 4 + 4, :]
                    rperm = bass.AP(tensor=rnat.tensor, offset=rnat.offset,
                                    ap=[rnat.ap[0], rnat.ap[1], [1, 8], [8, 16]])
                    nc.tensor.matmul(ps27, lhsT=wio_sb[:, ich, :], rhs=rperm,
                                     start=(ich == 0), stop=(ich == 1))
                ostg = initp.tile([27, 512], F32, tag="ostg", name="ostg",
                                  bufs=2)
                nc.scalar.activation(out=ostg,
                                     in_=ps27.rearrange("p a b -> p (a b)"),
                                     func=AF.Identity, bias=bio_sb, scale=1.0)
                ld(out=offcD[:, rg * 512:rg * 512 + 512], in_=ostg)

        # ================= phase B: DCN + heads ==============================
        wdc_sb = load_w("wdc", (128, 2, 9, 256), wdc)
        wdr_sb = load_w("wdr", (128, 2, 9, 256), wdr)
        wdcn = [wdc_sb, wdr_sb]

        with tc.tile_pool(name="bp", bufs=1) as bp, \
                tc.tile_pool(name="coefp", bufs=1) as coefp, \
                tc.tile_pool(name="pipep", bufs=2) as pipep, \
                tc.tile_pool(name="gthp", bufs=3) as gthp, \
                tc.tile_pool(name="wrkp", bufs=2) as wrkp, \
                tc.tile_pool(name="pB", bufs=1, space="PSUM") as pB:

            cf = []
            for hh in range(2):
                t = bp.tile([128, RF, WP], F32, tag=f"cf{hh}", name=f"cf{hh}")
                nc.vector.memset(t[:, :, 0:2], 0.0)
                nc.vector.memset(t[:, :, 130:132], 0.0)
                ld(out=t[:, :, 2:130], in_=cfD[hh])
                cf.append(t)
            feats = [cf, pf]

            def cw(name):
                return coefp.tile([9, FC], F32, tag=name, name=name)

            def sc():
                return coefp.tile([9, FC], F32, tag="sc", name="sc", bufs=4)

            for g in range(NCH):
                dy = pipep.tile([9, FC], F32, tag="dy", name="dy", bufs=1)
                dx = pipep.tile([9, FC], F32, tag="dx", name="dx", bufs=1)
                msk = pipep.tile([9, FC], F32, tag="msk", name="msk", bufs=1)
                ld(out=dy, in_=offcD[0:9, g * FC:(g + 1) * FC])
                ld(out=dx, in_=offcD[9:18, g * FC:(g + 1) * FC])
                ld(out=msk, in_=offcD[18:27, g * FC:(g + 1) * FC])
                rrow_g = pipep.tile([9, FC], F32, tag="rrowg", name="rrowg", bufs=1)
                ld(out=rrow_g, in_=rrow[:, g * FC:(g + 1) * FC])

                def floor_of(v, out_tag):
                    ti = cw(out_tag)
                    ti32 = coefp.tile([9, FC], I32, tag="i32", name="i32", bufs=1)
                    nc.vector.tensor_copy(ti32, v)
                    nc.vector.tensor_copy(ti, ti32)
                    gt_ = sc()
                    ttn(gt_, ti, v, OP.is_gt)
                    ttn(ti, ti, gt_, OP.subtract)
                    return ti

                py = sc()
                tsc(py, dy, kyc_sb, None, OP.add)
                ttn(py, py, rrow_g, OP.add)
                y0 = floor_of(py, "y0")
                fy = cw("fy")
                ttn(fy, py, y0, OP.subtract)
                px = sc()
                tsc(px, dx, kxc_sb, None, OP.add)
                ttn(px, px, wcol_sb, OP.add)
                x0 = floor_of(px, "x0")
                fx = cw("fx")
                ttn(fx, px, x0, OP.subtract)

                def wpair2(f, v, t0, t1):
                    # (w0, w1) = ((1-f)*valid(v), f*valid(v+1)) with sc scratch
                    c0 = sc()
                    tsc(c0, v, 127.0, 0.0, OP.min, OP.max)
                    v0 = sc()
                    ttn(v0, c0, v, OP.is_equal)
                    c1_ = sc()
                    tsc(c1_, v, 126.0, -1.0, OP.min, OP.max)
                    v1 = sc()
                    ttn(v1, c1_, v, OP.is_equal)
                    w1 = cw(t1)
                    ttn(w1, f, v1, OP.mult)
                    w0 = cw(t0)
                    tsc(w0, f, -1.0, 1.0, OP.mult, OP.add)
                    ttn(w0, w0, v0, OP.mult)
                    return w0, w1

                wy0, wy1 = wpair2(fy, y0, "wy0", "wy1")
                wx0, wx1 = wpair2(fx, x0, "wx0", "wx1")

                # gather indices (y0, x0 die here)
                ib = cw("ib")
                tsc(ib, y0, 132.0, None, OP.mult)
                ttn(ib, ib, x0, OP.add)
                tsc(ib, ib, cidx_sb, None, OP.add)
                idx16 = pipep.tile([9, 4, CHUNK, 128], I16, tag="idx16",
                                   name="idx16", bufs=1)
                for ci, dlt in enumerate((0.0, 1.0, 132.0, 133.0)):
                    snat = idx16[:, ci, :, :]
                    dst16 = bass.AP(tensor=snat.tensor, offset=snat.offset,
                                    ap=[snat.ap[0], snat.ap[1], [1, 8], [8, 16]])
                    if ci == 0:
                        nc.vector.tensor_copy(dst16, ib.rearrange(
                            "p (r m) -> p r m", m=128))
                    else:
                        tv = sc()
                        tsc(tv, ib, dlt, None, OP.add)
                        nc.vector.tensor_copy(dst16, tv.rearrange(
                            "p (r m) -> p r m", m=128))

                wref = []
                wcls = []
                for ci, (a_, b_) in enumerate(((wy0, wx0), (wy0, wx1), (wy1, wx0),
                                               (wy1, wx1))):
                    wr_ = pipep.tile([9, FC], F32, tag=f"wr{ci}", name=f"wr{ci}",
                                     bufs=1)
                    ttn(wr_, a_, b_, OP.mult)
                    wref.append(wr_)
                    wc_ = pipep.tile([9, FC], F32, tag=f"wc{ci}", name=f"wc{ci}",
                                     bufs=1)
                    ttn(wc_, wr_, msk, OP.mult)
                    wcls.append(wc_)
                dyb = pipep.tile([9, FC], F32, tag="dyb", name="dyb", bufs=1)
                tsc(dyb, dy, brop_sb[:, 0:1], None, OP.add)
                dxb = pipep.tile([9, FC], F32, tag="dxb", name="dxb", bufs=1)
                tsc(dxb, dx, brop_sb[:, 1:2], None, OP.add)

                # bounce via DRAM into 16-partition-wrapped layout
                bnc = dramp.tile([CHUNK, 16, 288], I16, tag="bnc", name="bnc")
                for ci in range(4):
                    for rl in range(CHUNK):
                        dst = bass.AP(
                            tensor=bnc.tensor,
                            offset=bnc.offset + rl * 16 * 288 + ci * 8,
                            ap=[[32, 9], [288, 16], [1, 8]])
                        ld(out=dst, in_=idx16[:, ci, rl, :])
                idxw = pipep.tile([128, 9, CHUNK, 32], I16, tag="idxw", name="idxw", bufs=1)
                for cc in range(8):
                    for rl in range(CHUNK):
                        srcb = bass.AP(tensor=bnc.tensor,
                                       offset=bnc.offset + rl * 16 * 288,
                                       ap=[[288, 16], [32, 9], [1, 32]])
                        ld(out=idxw[16 * cc:16 * cc + 16, :, rl, :], in_=srcb)

                clsb = pipep.tile([128, CHUNK, 80], F32, tag="clsb", name="clsb")

                # ---- rows of this chunk ----
                for rl in range(CHUNK):
                    r = g * CHUNK + rl
                    wcT = wrkp.tile([128, 3, 36], F32, tag="wcT", name="wcT")
                    sl = slice(rl * 128, (rl + 1) * 128)
                    for ti, group in enumerate((wref, wcls)):
                        pt = pB.tile([128, 128], F32, tag="ptr", name="ptr",
                                     bufs=2)
                        for ci in range(4):
                            nc.tensor.transpose(pt[:, 9 * ci:9 * ci + 9],
                                                group[ci][:, sl],
                                                ident_sb[:9, :9])
                        nc.scalar.copy(wcT[:, ti, :], pt[:, 0:36])
                    pt = pB.tile([128, 128], F32, tag="ptr", name="ptr", bufs=2)
                    nc.tensor.transpose(pt[:, 0:9], dyb[:, sl], ident_sb[:9, :9])
                    nc.tensor.transpose(pt[:, 9:18], dxb[:, sl], ident_sb[:9, :9])
                    nc.scalar.copy(wcT[:, 2, 0:18], pt[:, 0:18])

                    for d in (1, 0):
                        plane = 1 if d == 0 else 0
                        acc = wrkp.tile([128, 256], F32, tag=f"acc{d}",
                                        name=f"acc{d}")
                        for k in range(9):
                            gpair = []
                            for ich in range(2):
                                gt_ = gthp.tile([128, 512], F32, tag="g", name="g")
                                nc.gpsimd.ap_gather(
                                    out_ap=gt_,
                                    in_ap=feats[d][ich].rearrange(
                                        "p r w -> p (r w)"),
                                    idxs_ap=idxw[:, k, rl, :],
                                    channels=128, num_elems=NE, d=1, num_idxs=512)
                                gpair.append(gt_)
                            for ci in range(4):
                                G = pB.tile([128, 256], F32, tag="G", name="G",
                                            bufs=3)
                                nc.tensor.matmul(
                                    G, lhsT=gpair[0][:, ci * 128:ci * 128 + 128],
                                    rhs=wdcn[d][:, 0, k, :], start=True, stop=False)
                                nc.tensor.matmul(
                                    G, lhsT=gpair[1][:, ci * 128:ci * 128 + 128],
                                    rhs=wdcn[d][:, 1, k, :], start=False, stop=True)
                                scal = wcT[:, plane, 9 * ci + k:9 * ci + k + 1]
                                if k == 0 and ci == 0:
                                    tsc(acc, G, scal, None, OP.mult)
                                else:
                                    stt(acc, G, scal, acc, OP.mult, OP.add)
                        nc.scalar.activation(out=acc, in_=acc, func=AF.Relu)
                        rel = acc
                        dcnT = []
                        for ich in range(2):
                            ptx = pB.tile([128, 128], F32, tag="ptr", name="ptr2",
                                          bufs=2)
                            nc.tensor.transpose(ptx,
                                                rel[:, ich * 128:ich * 128 + 128],
                                                ident_sb)
                            dt_ = wrkp.tile([128, 128], F32, tag=f"dt{d}{ich}",
                                            name=f"dt{d}{ich}")
                            nc.scalar.copy(dt_, ptx)
                            dcnT.append(dt_)
                        if d == 0:
                            ph = pB.tile([128, 80], F32, tag="ph0", name="ph0",
                                         bufs=1)
                            for ich in range(2):
                                nc.tensor.matmul(ph, lhsT=dcnT[ich],
                                                 rhs=wco_sb[:, ich, :],
                                                 start=(ich == 0), stop=(ich == 1))
                            stt(clsb[:, rl, :], ph, 1.0, bco_sb, OP.mult, OP.add)
                        else:
                            ph = pB.tile([128, 18], F32, tag="ph1", name="ph1",
                                         bufs=1)
                            for ich in range(2):
                                nc.tensor.matmul(ph, lhsT=dcnT[ich],
                                                 rhs=wro_sb[:, ich, :],
                                                 start=(ich == 0), stop=(ich == 1))
                            ptr_t = wrkp.tile([128, 18], F32, tag="ptrt",
                                              name="ptrt")
                            stt(ptr_t, ph, 1.0, wcT[:, 2, 0:18], OP.mult,
                                OP.add)
                            for ax in range(2):
                                s_ = wrkp.tile([128, 1], F32, tag=f"sum{ax}",
                                               name=f"sum{ax}")
                                nc.vector.tensor_reduce(
                                    s_, ptr_t[:, ax * 9:ax * 9 + 9],
                                    axis=mybir.AxisListType.X, op=OP.add)
                                tsc(wrb[:, r, 2 + ax:3 + ax], s_, 1.0 / 9.0, None,
                                    OP.mult)
                                tadd = wrkp.tile([128, 9], F32, tag=f"ta{ax}",
                                                 name=f"ta{ax}")
                                tsc(tadd, ptr_t[:, ax * 9:ax * 9 + 9],
                                    wrb[:, r, 2 + ax:3 + ax], None, OP.add)
                                nc.vector.tensor_reduce(
                                    wrb[:, r, ax:ax + 1], tadd,
                                    axis=mybir.AxisListType.X, op=OP.max,
                                    apply_absolute_value=True)

                dst = bass.AP(tensor=co.tensor,
                              offset=co.offset + g * CHUNK * 128 * 80,
                              ap=[[80, 128], [128 * 80, CHUNK], [1, 80]])
                ld(out=dst, in_=clsb)

        dst = bass.AP(tensor=wr.tensor, offset=wr.offset,
                      ap=[[4, 128], [128 * 4, ROWS], [1, 4]])
        ld(out=dst, in_=wrb)

    nc.compile()
    return nc


# ---------------- host side ----------------

def prep_core_inputs(inputs, core):
    b, q = core // 4, core % 4
    h0 = q * ROWS
    d = {}
    x = np.asarray(inputs["x"])

    xsl = np.zeros((2, 128, RX, WP), np.float32)
    lo, hi = h0 - 6, h0 + ROWS + 6
    clo, chi = max(lo, 0), min(hi, H)
    xsl[:, :, clo - lo:chi - lo, 2:130] = \
        x[b].reshape(2, 128, H, W)[:, :, clo:chi, :]
    d["xs"] = xsl

    def tower_w(w):
        w = np.asarray(w)  # [oc, ic, 3, 3]
        t = w.transpose(1, 0, 2, 3).reshape(2, 128, 2, 128, 3, 3)
        return np.ascontiguousarray(
            t.transpose(1, 0, 2, 4, 5, 3)).reshape(128, 2, 2, 9, 128)

    for nm, src in (("w0c", "cls_w0"), ("w1c", "cls_w1"), ("w2c", "cls_w2"),
                    ("w0r", "reg_w0"), ("w1r", "reg_w1"), ("w2r", "reg_w2"),
                    ("wic", "init_conv_w")):
        d[nm] = tower_w(inputs[src])

    def dcn_w(w):
        w = np.asarray(w)
        t = w.transpose(1, 0, 2, 3).reshape(2, 128, 256, 3, 3)
        return np.ascontiguousarray(
            t.transpose(1, 0, 3, 4, 2)).reshape(128, 2, 9, 256)

    d["wdc"] = dcn_w(inputs["dcn_cls_w"])
    d["wdr"] = dcn_w(inputs["dcn_ref_w"])

    wio = np.asarray(inputs["init_out_w"])[:, :, 0, 0][PERM27]
    d["wio"] = np.ascontiguousarray(wio.T.reshape(2, 128, 27).transpose(1, 0, 2))
    wco = np.asarray(inputs["cls_out_w"])[:, :, 0, 0]
    d["wco"] = np.ascontiguousarray(wco.T.reshape(2, 128, 80).transpose(1, 0, 2))
    wro = np.asarray(inputs["ref_out_w"])[:, :, 0, 0][PERM18]
    d["wro"] = np.ascontiguousarray(wro.T.reshape(2, 128, 18).transpose(1, 0, 2))

    bt = np.zeros((128, 14), np.float32)
    for i, nm in enumerate(("cls_b0", "cls_b1", "cls_b2", "reg_b0", "reg_b1",
                            "reg_b2", "init_conv_b")):
        bb = np.asarray(inputs[nm])
        bt[:, 2 * i] = bb[:128]
        bt[:, 2 * i + 1] = bb[128:]
    d["bt"] = bt
    d["bio"] = np.asarray(inputs["init_out_b"])[PERM27].reshape(27, 1).astype(
        np.float32)
    d["bco"] = np.tile(np.asarray(inputs["cls_out_b"])[None, :], (128, 1)).astype(
        np.float32)
    brp = np.asarray(inputs["ref_out_b"])[PERM18].astype(np.float32)
    d["brop"] = np.stack([brp[0:9], brp[9:18]], axis=1)

    ky = (np.arange(9) // 3 - 1).astype(np.float32)
    kx = (np.arange(9) % 3 - 1).astype(np.float32)
    d["kyc"] = ky.reshape(9, 1)
    d["kxc"] = kx.reshape(9, 1)
    d["cidx"] = np.full((9, 1), 2.0 - 132.0 * (h0 - 3), np.float32)
    rr = np.repeat(np.arange(ROWS, dtype=np.float32) + h0, 128)
    d["rrow"] = np.tile(rr[None, :], (9, 1))
    pi = np.array([(m % 16) * 8 + m // 16 for m in range(128)], np.int64)
    wc = np.tile(pi.astype(np.float32), CHUNK)
    d["wcol"] = np.tile(wc[None, :], (9, 1))
    d["ident"] = np.eye(128, dtype=np.float32)

    mr = np.zeros((128, 24), np.float32)
    cols = [(0, R1, 5, 5), (10, R2, 4, 4), (18, RF, 3, 3)]
    for c0, rtot, off, m in cols:
        for j in range(m):
            mr[:, c0 + j] = 1.0 if 0 <= (h0 - off + j) < H else 0.0
            mr[:, c0 + m + j] = 1.0 if 0 <= (h0 - off + rtot - m + j) < H else 0.0
    d["mrow"] = mr
    return d


_PROG = None


def _get_prog():
    global _PROG
    if _PROG is None:
        _PROG = build_program_flat()
    return _PROG


def kernel(**inputs):
    nc = _get_prog()
    in_maps = [prep_core_inputs(inputs, c) for c in range(N_CORES)]
    res = run_bass_kernel_spmd(nc, in_maps, list(range(N_CORES))).results
    cls_out = np.zeros((B, NCLS, H, W), np.float32)
    wh = np.zeros((B, 2, H, W), np.float32)
    reg = np.zeros((B, 2, H, W), np.float32)
    for c in range(N_CORES):
        b, q = c // 4, c % 4
        h0 = q * ROWS
        PI = np.array([(m % 16) * 8 + m // 16 for m in range(128)], np.int64)
        PIINV = np.argsort(PI)
        cls_out[b, :, h0:h0 + ROWS, :] = \
            res[c]["co"].reshape(ROWS, 128, NCLS)[:, PIINV, :].transpose(2, 0, 1)
        w4 = res[c]["wr"].reshape(ROWS, 128, 4)[:, PIINV, :]
        wh[b, 0, h0:h0 + ROWS] = w4[:, :, 0]
        wh[b, 1, h0:h0 + ROWS] = w4[:, :, 1]
        reg[b, 0, h0:h0 + ROWS] = w4[:, :, 2]
        reg[b, 1, h0:h0 + ROWS] = w4[:, :, 3]
    return cls_out, wh, reg
